# revision 14
# baseline (speedup 1.0000x reference)
"""AttentionDecoder step kernel for Trainium2 (8 NeuronCores, batch-parallel).

B=128 batch is sharded 8 ways (16 rows/core). All weights replicated.
Heavy math (keys projection, score reduction, context) runs in bf16 on the PE;
GRU chain + softmax run in fp32. Hidden-state inputs are zeros per the problem
spec (fill: zeros), so gh = bhh for every GRU cell.

Per-core dataflow (B_loc = 16, T = 1024, U = 256):
  - memory[b] DMA-loaded f32->bf16 (SWDGE cast) into natural [t,u] tiles,
    then xbar DMA-transposed to memT [u, t] tiles.
  - keysT = W1 @ memT on PE (bf16); tanh(.+query) on ACT -> tanhK (bf16).
  - scores = vT . tanhK (PE, M=1) col-tiled into psum rows {0,32,64,96} so
    softmax runs batched (4 batch rows per psum group).
  - softmax on DVE/ACT; attn weights xbar-transposed to columns.
  - context = attn @ memory (PE, col-tiled output rows).
  - GRU chain in transposed layout [feature-on-partition, b-free]; PE
    transposes (identity-matmuls) restore natural layout for outputs.
"""

import numpy as np
import ml_dtypes
from contextlib import ExitStack

B = 128
T = 1024
U = 256
IN = 128
NMEL = 400  # 80 * 5
B_LOC = 16
N_CORES = 8
NT = T // 128  # 8 t-tiles per batch row

_CACHE = {}


def _host_prep(W1, W2, v, attn_Wih, attn_bih, attn_bhh, g1_Wih, g1_bih, g1_bhh,
               g2_Wih, g2_bih, g2_bhh, proj_W, proj_b, out_W, out_b):
    bf16 = ml_dtypes.bfloat16
    f32 = np.float32
    w1t = np.ascontiguousarray(np.asarray(W1, f32).T).reshape(2, 128, 256).astype(bf16)
    vt = np.ascontiguousarray(np.asarray(v, f32)[0]).reshape(2, 128, 1).astype(bf16)
    w2t = np.ascontiguousarray(np.asarray(W2, f32).T).reshape(2, 128, 256).astype(f32)
    awih_t = np.ascontiguousarray(np.asarray(attn_Wih, f32).T).astype(f32)
    g1wih_t = np.ascontiguousarray(np.asarray(g1_Wih, f32).T).reshape(2, 128, 768).astype(f32)
    g2wih_t = np.ascontiguousarray(np.asarray(g2_Wih, f32).T).reshape(2, 128, 768).astype(f32)
    projt = np.ascontiguousarray(np.asarray(proj_W, f32).T).reshape(4, 128, 256).astype(f32)
    outwt = np.ascontiguousarray(np.asarray(out_W, f32).T).reshape(2, 128, 400).astype(f32)

    cols = []
    for bih, bhh in ((attn_bih, attn_bhh), (g1_bih, g1_bhh), (g2_bih, g2_bhh)):
        bih = np.asarray(bih, f32)
        bhh = np.asarray(bhh, f32)
        comb = bih + bhh
        cols += [comb[0:128], comb[128:256], comb[256:384], comb[384:512]]
        cols += [bih[512:640], bih[640:768], bhh[512:640], bhh[640:768]]
    proj_b = np.asarray(proj_b, f32)
    cols += [proj_b[0:128], proj_b[128:256]]
    ob = np.zeros(512, f32)
    ob[:400] = np.asarray(out_b, f32)
    cols += [ob[0:128], ob[128:256], ob[256:384], ob[384:512]]
    biases = np.ascontiguousarray(np.stack(cols, axis=1))  # [128, 30]
    ident = np.eye(128, dtype=f32)
    identb = np.eye(128, dtype=bf16)
    return dict(w1t=w1t, vt=vt, w2t=w2t, awih_t=awih_t, g1wih_t=g1wih_t,
                g2wih_t=g2wih_t, projt=projt, outwt=outwt, biases=biases,
                ident=ident, identb=identb)


def _build():
    import concourse.bass as bass
    import concourse.tile as tile
    from concourse import bacc, mybir

    f32 = mybir.dt.float32
    bf16 = mybir.dt.bfloat16
    AF = mybir.ActivationFunctionType
    ALU = mybir.AluOpType
    AX = mybir.AxisListType

    nc = bacc.Bacc("TRN2", target_bir_lowering=False, debug=False,
                   num_devices=N_CORES)

    din = nc.dram_tensor("dec_x", [B_LOC, IN], f32, kind="ExternalInput").ap()
    # memory pre-cast to bf16 on host, u-half-major: [b, uh, t, u_in_half]
    dmem = nc.dram_tensor("mem", [B_LOC, 2, T, 128], bf16,
                          kind="ExternalInput").ap()
    dw1t = nc.dram_tensor("w1t", [2, 128, 256], bf16, kind="ExternalInput").ap()
    dvt = nc.dram_tensor("vt", [2, 128, 1], bf16, kind="ExternalInput").ap()
    dw2t = nc.dram_tensor("w2t", [2, 128, 256], f32, kind="ExternalInput").ap()
    dawih = nc.dram_tensor("awih_t", [128, 768], f32, kind="ExternalInput").ap()
    dg1wih = nc.dram_tensor("g1wih_t", [2, 128, 768], f32, kind="ExternalInput").ap()
    dg2wih = nc.dram_tensor("g2wih_t", [2, 128, 768], f32, kind="ExternalInput").ap()
    dprojt = nc.dram_tensor("projt", [4, 128, 256], f32, kind="ExternalInput").ap()
    doutwt = nc.dram_tensor("outwt", [2, 128, 400], f32, kind="ExternalInput").ap()
    dbias = nc.dram_tensor("biases", [128, 30], f32, kind="ExternalInput").ap()
    dident = nc.dram_tensor("ident", [128, 128], f32, kind="ExternalInput").ap()
    didentb = nc.dram_tensor("identb", [128, 128], bf16, kind="ExternalInput").ap()

    dmel = nc.dram_tensor("mel", [B_LOC, NMEL], f32, kind="ExternalOutput").ap()
    ddt = nc.dram_tensor("dt_out", [B_LOC, U], f32, kind="ExternalOutput").ap()
    dg1h = nc.dram_tensor("g1h_out", [B_LOC, U], f32, kind="ExternalOutput").ap()
    dg2h = nc.dram_tensor("g2h_out", [B_LOC, U], f32, kind="ExternalOutput").ap()

    with tile.TileContext(nc) as tc, ExitStack() as ctx:
        consts = ctx.enter_context(tc.tile_pool(name="consts", bufs=1))
        mempool = ctx.enter_context(tc.tile_pool(name="mem", bufs=1))
        mtp = ctx.enter_context(tc.tile_pool(name="mtp", bufs=3))
        tkp = ctx.enter_context(tc.tile_pool(name="tkp", bufs=3))
        smp = ctx.enter_context(tc.tile_pool(name="smp", bufs=2))
        atp = ctx.enter_context(tc.tile_pool(name="atp", bufs=4))
        outp = ctx.enter_context(tc.tile_pool(name="outp", bufs=2))
        # PSUM budget (8 banks): kps 2 + scores 2 + ctx 1 + small 2 = 7
        kpsp = ctx.enter_context(tc.tile_pool(name="kpsp", bufs=2, space="PSUM"))
        scpsp = ctx.enter_context(tc.tile_pool(name="scpsp", bufs=1, space="PSUM"))
        ctxpsp = ctx.enter_context(tc.tile_pool(name="ctxpsp", bufs=1, space="PSUM"))
        spsp = ctx.enter_context(tc.tile_pool(name="spsp", bufs=2, space="PSUM"))

        # ---- constants to SBUF ----
        w1t_sb = consts.tile([128, 2, 256], bf16)
        nc.sync.dma_start(w1t_sb, dw1t.rearrange("k p v -> p k v"))
        vt_sb = consts.tile([128, 2, 1], bf16)
        nc.sync.dma_start(vt_sb, dvt.rearrange("k p v -> p k v"))
        w2t_sb = consts.tile([128, 2, 256], f32)
        nc.sync.dma_start(w2t_sb, dw2t.rearrange("k p v -> p k v"))
        awih_sb = consts.tile([128, 768], f32)
        nc.sync.dma_start(awih_sb, dawih)
        g1wih_sb = consts.tile([128, 2, 768], f32)
        nc.sync.dma_start(g1wih_sb, dg1wih.rearrange("k p v -> p k v"))
        g2wih_sb = consts.tile([128, 2, 768], f32)
        nc.sync.dma_start(g2wih_sb, dg2wih.rearrange("k p v -> p k v"))
        projt_sb = consts.tile([128, 4, 256], f32)
        nc.sync.dma_start(projt_sb, dprojt.rearrange("k p v -> p k v"))
        outwt_sb = consts.tile([128, 2, 400], f32)
        nc.sync.dma_start(outwt_sb, doutwt.rearrange("k p v -> p k v"))
        bias_sb = consts.tile([128, 30], f32)
        nc.sync.dma_start(bias_sb, dbias)
        ident_sb = consts.tile([128, 128], f32)
        nc.sync.dma_start(ident_sb, dident)
        identb_sb = consts.tile([128, 128], bf16)
        nc.sync.dma_start(identb_sb, didentb)

        def mm(out, lhsT, rhs, start, stop, tp=None):
            nc.tensor.matmul(out, lhsT, rhs, start=start, stop=stop,
                             tile_position=tp)

        # GRU gates helper (transposed layout): inT [128, nk, 16] f32 -> hT.
        # Gate cols at cb: 0-3 = (bih+bhh) r,z; 4-5 = bih_n; 6-7 = bhh_n.
        def gru(name, inT, wih_sb, nk, cb):
            gates = smp.tile([128, 6, 16], f32, tag=f"gates_{name}")
            for g in range(6):
                gps = spsp.tile([128, 16], f32, tag="sps")
                for k in range(nk):
                    lhs = wih_sb[:, k, g * 128:(g + 1) * 128] if nk > 1 \
                        else wih_sb[:, g * 128:(g + 1) * 128]
                    mm(gps, lhs, inT[:, k, :], k == 0, k == nk - 1)
                func = AF.Sigmoid if g < 4 else AF.Identity
                nc.scalar.activation(gates[:, g, :], gps, func,
                                     bias=bias_sb[:, cb + g:cb + g + 1])
            hT = smp.tile([128, 2, 16], f32, tag=f"hT_{name}")
            for j in range(2):
                t1 = smp.tile([128, 16], f32, tag="t1")
                nc.vector.tensor_scalar_mul(t1, gates[:, j, :],
                                            bias_sb[:, cb + 6 + j:cb + 7 + j])
                t2 = smp.tile([128, 16], f32, tag="t2")
                nc.vector.tensor_add(t2, t1, gates[:, 4 + j, :])
                ntile = smp.tile([128, 16], f32, tag="ntile")
                nc.scalar.activation(ntile, t2, AF.Tanh)
                zn = smp.tile([128, 16], f32, tag="zn")
                nc.vector.tensor_mul(zn, gates[:, 2 + j, :], ntile)
                nc.vector.tensor_sub(hT[:, j, :], ntile, zn)
            return hT

        # transpose back to natural [16, F] and DMA out
        def untranspose_out(slices, nat_tile, dram_out):
            off = 0
            for sl in slices:
                rows = sl.partition_size()
                tps2 = spsp.tile([16, 128], f32, tag="sps")
                mm(tps2[:16, :rows], sl, ident_sb[:rows, :rows], True, True)
                nc.scalar.copy(nat_tile[:, off:off + rows], tps2[:16, :rows])
                off += rows
            nc.sync.dma_start(dram_out, nat_tile)

        # ---- small chain part 1: attn GRU + query ----
        decx_sb = smp.tile([16, 128], f32, tag="decx")
        nc.sync.dma_start(decx_sb, din)
        xps = spsp.tile([128, 16], f32, tag="sps")
        mm(xps, decx_sb, ident_sb[:16, :16], True, True)  # -> xT [128, 16]
        xT = smp.tile([128, 1, 16], f32, tag="xT")
        nc.scalar.copy(xT[:, 0, :], xps)

        dtT = gru("a", xT, awih_sb, 1, 0)

        qT_sb = smp.tile([128, 2, 16], f32, tag="qT")
        for m in range(2):
            qps = spsp.tile([128, 16], f32, tag="sps")
            for k in range(2):
                mm(qps, w2t_sb[:, k, m * 128:(m + 1) * 128], dtT[:, k, :],
                   k == 0, k == 1)
            nc.scalar.copy(qT_sb[:, m, :], qps)

        dt_nat = outp.tile([16, 256], f32, tag="dt_nat")
        untranspose_out([dtT[:, 0, :], dtT[:, 1, :]], dt_nat, ddt)

        # ---- big pipeline ----
        mem_nat = [mempool.tile([128, 2, NT, 128], bf16, tag=f"mem{b}",
                                name=f"mem_nat{b}")
                   for b in range(B_LOC)]
        dmem_r = dmem.rearrange("bb uh (nt p) u -> bb p uh nt u", p=128)

        sc_tiles = [None] * 4
        attnT_tiles = [None] * 4

        for b in range(B_LOC):
            nc.sync.dma_start(mem_nat[b], dmem_r[b])

            memT = mtp.tile([128, 2, 1024], bf16, tag="memT")
            for uh in range(2):
                # xbar transpose straight from DRAM (no wait -> walrus-safe)
                nc.sync.dma_start_transpose(memT[:, uh, :], dmem[b, uh])

            tk = tkp.tile([128, 2, 1024], bf16, tag="tk")
            for vc in range(2):
                for tc_ in range(2):
                    kps = kpsp.tile([128, 512], f32, tag="kps")
                    for k in range(2):
                        mm(kps, w1t_sb[:, k, vc * 128:(vc + 1) * 128],
                           memT[:, k, tc_ * 512:(tc_ + 1) * 512], k == 0, k == 1)
                    nc.scalar.activation(tk[:, vc, tc_ * 512:(tc_ + 1) * 512],
                                         kps, AF.Tanh,
                                         bias=qT_sb[:, vc, b:b + 1])

            g, j = divmod(b, 4)
            if j == 0:
                sc_tiles[g] = scpsp.tile([128, 1024], f32, tag="scps",
                                         name=f"scps{g}")
                nc.vector.memset(sc_tiles[g], 0.0)
            sc_ps = sc_tiles[g]
            for tc_ in range(2):
                for vc in range(2):
                    mm(sc_ps[32 * j:32 * j + 1, tc_ * 512:(tc_ + 1) * 512],
                       vt_sb[:, vc, :], tk[:, vc, tc_ * 512:(tc_ + 1) * 512],
                       vc == 0, vc == 1, tp=(0, 32 * j))

            if j == 3:
                # batched softmax for rows {0,32,64,96}; other rows are junk
                mx = smp.tile([128, 1], f32, tag="mx")
                nc.vector.tensor_reduce(mx, sc_ps, axis=AX.X, op=ALU.max)
                nmx = smp.tile([128, 1], f32, tag="nmx")
                nc.vector.tensor_scalar_mul(nmx, mx, -1.0)
                expw = atp.tile([128, 1024], f32, tag="expw")
                sume = smp.tile([128, 1], f32, tag="sume")
                nc.scalar.activation(expw, sc_ps, AF.Exp, bias=nmx,
                                     accum_out=sume)
                rs = smp.tile([128, 1], f32, tag="rs")
                nc.vector.reciprocal(rs, sume)
                abf = atp.tile([128, 1024], bf16, tag="abf")
                nc.vector.tensor_scalar_mul(abf, expw, rs)

                atT = atp.tile([128, NT, 128], bf16, tag="atT")
                for nt in range(NT):
                    aps_ = spsp.tile([128, 128], f32, tag="sps")
                    mm(aps_, abf[:, nt * 128:(nt + 1) * 128], identb_sb,
                       True, True)
                    nc.scalar.copy(atT[:, nt, :], aps_)
                attnT_tiles[g] = atT

        # ---- context (col-tiled psum rows) ----
        ctxT = smp.tile([128, 2, 16], f32, tag="ctxT")
        for g in range(4):
            ctx_ps = ctxpsp.tile([128, 256], f32, tag="ctxps")
            for j in range(4):
                b = 4 * g + j
                for nt in range(NT):
                    mm(ctx_ps[32 * j:32 * j + 1, :],
                       attnT_tiles[g][:, nt, 32 * j:32 * j + 1],
                       mem_nat[b][:, :, nt, :], nt == 0, nt == NT - 1,
                       tp=(0, 32 * j))
            ctx_sb = smp.tile([128, 256], f32, tag="ctx_sb")
            nc.scalar.copy(ctx_sb, ctx_ps)
            for uh in range(2):
                cps2 = spsp.tile([128, 128], f32, tag="sps")
                mm(cps2, ctx_sb[:, uh * 128:(uh + 1) * 128], ident_sb,
                   True, True)
                for j in range(4):
                    nc.scalar.copy(ctxT[:, uh, 4 * g + j:4 * g + j + 1],
                                   cps2[:, 32 * j:32 * j + 1])

        # ---- gru1 / gru2 / output ----
        g1inT = smp.tile([128, 2, 16], f32, tag="g1inT")
        for m in range(2):
            ips = spsp.tile([128, 16], f32, tag="sps")
            for k in range(4):
                rhs = dtT[:, k, :] if k < 2 else ctxT[:, k - 2, :]
                mm(ips, projt_sb[:, k, m * 128:(m + 1) * 128], rhs,
                   k == 0, k == 3)
            nc.scalar.activation(g1inT[:, m, :], ips, AF.Identity,
                                 bias=bias_sb[:, 24 + m:25 + m])

        g1hT = gru("g1", g1inT, g1wih_sb, 2, 8)
        g1h_nat = outp.tile([16, 256], f32, tag="g1h_nat")
        untranspose_out([g1hT[:, 0, :], g1hT[:, 1, :]], g1h_nat, dg1h)

        g2inT = smp.tile([128, 2, 16], f32, tag="g2inT")
        for k in range(2):
            nc.vector.tensor_add(g2inT[:, k, :], g1inT[:, k, :], g1hT[:, k, :])
        g2hT = gru("g2", g2inT, g2wih_sb, 2, 16)
        g2h_nat = outp.tile([16, 256], f32, tag="g2h_nat")
        untranspose_out([g2hT[:, 0, :], g2hT[:, 1, :]], g2h_nat, dg2h)

        bfT = smp.tile([128, 2, 16], f32, tag="bfT")
        for k in range(2):
            nc.vector.tensor_add(bfT[:, k, :], g2inT[:, k, :], g2hT[:, k, :])

        melT = smp.tile([128, 4, 16], f32, tag="melT")
        msizes = [(0, 128), (128, 128), (256, 128), (384, 16)]
        for m, (off, rows) in enumerate(msizes):
            ops_ = spsp.tile([128, 16], f32, tag="sps")
            for k in range(2):
                mm(ops_[:rows, :], outwt_sb[:, k, off:off + rows], bfT[:, k, :],
                   k == 0, k == 1)
            nc.scalar.activation(melT[:rows, m, :], ops_[:rows, :], AF.Identity,
                                 bias=bias_sb[:rows, 26 + m:27 + m])
        mel_nat = outp.tile([16, NMEL], f32, tag="mel_nat")
        untranspose_out([melT[:, 0, :], melT[:, 1, :], melT[:, 2, :],
                         melT[:16, 3, :]], mel_nat, dmel)

    nc.finalize()  # run Bacc compile passes (wait splitting, events, ...)
    return nc


def _get_nc():
    if "nc" not in _CACHE:
        _CACHE["nc"] = _build()
    return _CACHE["nc"]


def kernel(**inputs):
    from concourse.bass_utils import run_bass_kernel_spmd

    for hname in ("attn_hidden", "gru1_hidden", "gru2_hidden"):
        h = inputs.get(hname)
        if h is not None:
            assert float(np.abs(np.asarray(h)).max()) == 0.0, \
                f"{hname} must be zeros (problem spec fill=zeros)"

    prep = _host_prep(
        inputs["W1"], inputs["W2"], inputs["v"],
        inputs["attn_Wih"], inputs["attn_bih"], inputs["attn_bhh"],
        inputs["g1_Wih"], inputs["g1_bih"], inputs["g1_bhh"],
        inputs["g2_Wih"], inputs["g2_bih"], inputs["g2_bhh"],
        inputs["proj_W"], inputs["proj_b"], inputs["out_W"], inputs["out_b"])

    dec = np.asarray(inputs["decoder_input"], np.float32)
    mem = np.asarray(inputs["memory"], np.float32)
    # bf16 cast + u-half-major layout [B, 2, T, 128] (contiguous xbar source)
    memb = np.ascontiguousarray(
        mem.astype(ml_dtypes.bfloat16).reshape(B, T, 2, 128).transpose(0, 2, 1, 3))

    nc = _get_nc()
    in_maps = []
    for c in range(N_CORES):
        s = slice(c * B_LOC, (c + 1) * B_LOC)
        m = {"dec_x": np.ascontiguousarray(dec[s]),
             "mem": memb[s]}
        m.update(prep)
        in_maps.append(m)

    res = run_bass_kernel_spmd(nc, in_maps, core_ids=list(range(N_CORES)),
                               **_CACHE.get("run_kwargs", {}))
    _CACHE["last_result"] = res
    outs = res.results
    mel = np.concatenate([o["mel"] for o in outs], axis=0)
    d_t = np.concatenate([o["dt_out"] for o in outs], axis=0)
    g1h = np.concatenate([o["g1h_out"] for o in outs], axis=0)
    g2h = np.concatenate([o["g2h_out"] for o in outs], axis=0)
    output = mel.reshape(B, 80, 5).astype(np.float32)
    return (output, d_t.astype(np.float32), g1h.astype(np.float32),
            g2h.astype(np.float32))


if __name__ == "__main__":
    nc = _get_nc()
    print("bass build OK")
